# revision 2
# baseline (speedup 1.0000x reference)
"""Trainium2 Bass kernel for the MFA/MPPCA mixture log-likelihood problem.

Math: out[n,k] = PI[k] + logprob[n,k] with Sigma_k = A_k A_k^T + diag(D_k^2),
computed via Woodbury.  Everything involving only the small parameters
(MU, A, D, PI) is folded on the host into:

    out[n,k] = CONST[k] + x[n]·H[:,k] + (x[n]^2)·G[:,k] + sum_l (x[n]·Csc[:,k,l])^2

where (with iD = D^-2, B = iD*A, L = I + A^T B, iL = inv(L), R = chol(iL),
C0 = B R, e = R^T B^T MU):
    G   = -0.5 * iD^T                       (d, K)
    H   = (iD*MU)^T - C0 e                  (d, K)
    Csc = sqrt(0.5) * C0                    (d, K*l)
    CONST = PI - 0.5*(d log 2pi + logdet Sigma + MU^T iD MU) + 0.5 |e|^2

Device kernel (data-parallel over N on 8 cores): x and x^2 are packed on the
host into one fp16 stream per 128-sample tile (8 chunks of 128 features:
x c0..c3, x^2 c4..c7) so each tile needs exactly one 2KB-per-partition DMA.
PE accumulates [H+G | Csc] projections into a single 2-psum-bank tile
(cols 0:504 and 512:712), ScalarE squares the 640 factor projections into a
[128, 64, 11] buffer whose 11th lane is pre-filled with CONST[k], VectorE
does one group-of-11 reduce (q2 + CONST in one op) and one final add with
psum[:, 0:64] (H+G).  Output DMA is batched 4 tiles at a time.  A short
garbage-matmul warmup keeps the PE HAM clock-gate at 2.4GHz from the start.
"""
import math
import numpy as np

N_TOTAL, K, D_FEAT, L_FAC = 131072, 64, 512, 10
N_CORES = 8
N_PER_CORE = N_TOTAL // N_CORES  # 16384

WALL_COLS = K + K * L_FAC          # 704 = [H (0:64) | Csc (64:704)]
ACOL = 504                         # a-slice: H + 44 groups  (psum cols 0:504)
BCOL = WALL_COLS - ACOL            # 200 = 20 groups         (psum cols 512:712)
NGA = (ACOL - K) // L_FAC          # 44
NGB = BCOL // L_FAC                # 20
TILES = N_PER_CORE // 128          # 128
DMA_BATCH = 4                      # sample tiles per input/output DMA


def host_prep(MU, A, D, PI):
    """Fold small-parameter math into matmul weights (float64 internally)."""
    MU64, A64, D64, PI64 = [np.asarray(v, np.float64) for v in (MU, A, D, PI)]
    Kc, d, l = A64.shape
    iD = D64 ** -2.0
    B = iD[..., None] * A64
    L = np.eye(l)[None] + np.einsum('kdl,kdm->klm', A64, B)
    sign, logdet_L = np.linalg.slogdet(L)
    log_det_Sigma = logdet_L - np.sum(np.log(iD), axis=1)
    iL = np.linalg.inv(L)
    R = np.linalg.cholesky(iL)                  # R @ R.T = iL
    C0 = np.einsum('kdl,klm->kdm', B, R)        # (K, d, l)
    bmu = np.einsum('kdl,kd->kl', B, MU64)
    e = np.einsum('klm,kl->km', R, bmu)         # (K, l)
    c1 = np.sum(iD * MU64 * MU64, axis=1)

    CONST = PI64 - 0.5 * (d * math.log(2.0 * math.pi) + log_det_Sigma + c1) \
        + 0.5 * np.sum(e * e, axis=1)
    G = (-0.5 * iD).T
    H = (iD * MU64 - np.einsum('kdm,km->kd', C0, e)).T
    Csc = (C0 * np.sqrt(0.5)).transpose(1, 0, 2).reshape(d, Kc * l)  # k-major

    wall = np.concatenate([H, Csc], axis=1).astype(np.float16)     # (d, 704)
    g16 = G.astype(np.float16)                                      # (d, K)
    ctile = np.tile(CONST.astype(np.float32)[None, :], (128, 1))    # (128, K)
    return wall, g16, ctile


def pack_core_input(xs):
    """xs: (n_per_core, 512) fp32 -> (TILES, 128*8*128) fp16 tile-major pack.

    Per tile t, partition p: 8 chunks of 128 contiguous fp16 values:
    chunks 0:4 = x[d = c*128+p, t*128:(t+1)*128], chunks 4:8 = x^2 likewise.
    """
    n = xs.shape[0]
    x16 = xs.T.astype(np.float16)                      # (512, n)
    x2 = (x16 * x16).astype(np.float16)                # exact squares of fp16 x
    # (512, n) -> [c, p, t, j] -> [t, p, c, j]
    xr = x16.reshape(4, 128, n // 128, 128).transpose(2, 1, 0, 3)
    x2r = x2.reshape(4, 128, n // 128, 128).transpose(2, 1, 0, 3)
    packed = np.concatenate([xr, x2r], axis=2)         # (t, 128, 8, 128)
    return np.ascontiguousarray(packed).reshape(n // 128, 128 * 8 * 128)


def build_nc(n_per_core=N_PER_CORE):
    """Build and compile the Bass module for one core (SPMD across 8)."""
    import concourse.bacc as bacc
    import concourse.tile as tile
    import concourse.mybir as mybir

    f32 = mybir.dt.float32
    f16 = mybir.dt.float16
    assert n_per_core % (128 * DMA_BATCH) == 0
    tiles = n_per_core // 128
    nbatch = tiles // DMA_BATCH

    nc = bacc.Bacc("TRN2", target_bir_lowering=False, debug=False,
                   enable_asserts=False, num_devices=N_CORES)
    xx2_dram = nc.dram_tensor("xx2", (tiles, 128 * 8 * 128), f16,
                              kind="ExternalInput")
    wall_dram = nc.dram_tensor("wall", (D_FEAT, WALL_COLS), f16,
                               kind="ExternalInput")
    g_dram = nc.dram_tensor("g16", (D_FEAT, K), f16, kind="ExternalInput")
    c_dram = nc.dram_tensor("ctile", (128, K), f32, kind="ExternalInput")
    out_dram = nc.dram_tensor("out", (n_per_core, K), f32, kind="ExternalOutput")

    xx2_v = xx2_dram.ap().rearrange("t (p c j) -> t p c j", p=128, c=8)
    wall_v = wall_dram.ap().rearrange("(c p) m -> p c m", p=128)  # [128, 4, 704]
    g_v = g_dram.ap().rearrange("(c p) m -> p c m", p=128)        # [128, 4, 64]
    out_v = out_dram.ap().rearrange("(b u p) k -> b p u k", p=128, u=DMA_BATCH)

    with tile.TileContext(nc) as tc:
        with (
            tc.tile_pool(name="wpool", bufs=1) as wpool,
            tc.tile_pool(name="xpool", bufs=3) as xpool,
            tc.tile_pool(name="spool", bufs=3) as spool,
            tc.tile_pool(name="opool", bufs=2) as opool,
            tc.tile_pool(name="ppool", bufs=3, space="PSUM") as ppool,
            tc.tile_pool(name="wmpool", bufs=1, space="PSUM") as wmpool,
        ):
            # --- HAM warmup: keep PE busy while the first DMAs land ---
            warm = wpool.tile([128, 256], f16)
            nc.vector.memset(warm[:], 0.0)
            wpsum = wmpool.tile([128, 256], f32)
            NWARM = 10
            for j in range(NWARM):
                nc.tensor.matmul(wpsum[:], warm[:, 0:128], warm[:],
                                 start=(j == 0), stop=(j == NWARM - 1))

            wall_sb = wpool.tile([128, 4, WALL_COLS], f16)
            nc.sync.dma_start(out=wall_sb[:], in_=wall_v[:])
            g_sb = wpool.tile([128, 4, K], f16)
            nc.sync.dma_start(out=g_sb[:], in_=g_v[:])
            c_sb = wpool.tile([128, K], f32)
            nc.sync.dma_start(out=c_sb[:], in_=c_dram.ap())

            # two squares buffers (ping-pong); lane 10 of each group holds
            # CONST[k] so the group reduce emits q2 + CONST directly
            sq0 = wpool.tile([128, K, L_FAC + 1], f32)
            sq1 = wpool.tile([128, K, L_FAC + 1], f32)
            nc.vector.tensor_copy(sq0[:, :, L_FAC], c_sb[:])
            nc.vector.tensor_copy(sq1[:, :, L_FAC], c_sb[:])
            sqs = (sq0, sq1)

            for b in range(nbatch):
                xb = xpool.tile([128, DMA_BATCH, 8, 128], f16, tag="xb")
                nc.sync.dma_start(out=xb[:], in_=xx2_v[b * DMA_BATCH:
                                                       (b + 1) * DMA_BATCH]
                                  .rearrange("t p c j -> p t c j"))
                ob = opool.tile([128, DMA_BATCH, K], f32, tag="ob")
                for u in range(DMA_BATCH):
                    t = b * DMA_BATCH + u
                    psum = ppool.tile([128, 768], f32, tag="ps")

                    def mm_a(c, start, stop):
                        nc.tensor.matmul(psum[:, 0:ACOL], xb[:, u, c, :],
                                         wall_sb[:, c, 0:ACOL],
                                         start=start, stop=stop)

                    def mm_b(c, start, stop):
                        nc.tensor.matmul(psum[:, 512:512 + BCOL], xb[:, u, c, :],
                                         wall_sb[:, c, ACOL:WALL_COLS],
                                         start=start, stop=stop)

                    mm_a(0, True, False)
                    mm_b(0, True, False)
                    mm_a(1, False, False)
                    mm_b(1, False, False)
                    mm_a(2, False, False)
                    mm_b(2, False, False)
                    for c in range(4):
                        nc.tensor.matmul(psum[:, 0:K], xb[:, u, 4 + c, :],
                                         g_sb[:, c, :], start=False, stop=False,
                                         skip_group_check=True)
                    mm_a(3, False, True)
                    mm_b(3, False, True)

                    sq = sqs[t % 2]
                    nc.scalar.square(
                        sq[:, 0:NGA, 0:L_FAC],
                        psum[:, K:ACOL].rearrange("p (g t) -> p g t", t=L_FAC))
                    nc.scalar.square(
                        sq[:, NGA:K, 0:L_FAC],
                        psum[:, 512:512 + BCOL]
                        .rearrange("p (g t) -> p g t", t=L_FAC))

                    red = spool.tile([128, K], f32, tag="red")
                    nc.vector.reduce_sum(red[:], sq[:],
                                         axis=mybir.AxisListType.X)
                    nc.vector.tensor_add(ob[:, u, :], red[:], psum[:, 0:K])

                nc.sync.dma_start(out=out_v[b], in_=ob[:])

    nc.compile()
    return nc


_NC_CACHE = {}


def _get_nc(n_per_core=N_PER_CORE):
    if n_per_core not in _NC_CACHE:
        _NC_CACHE[n_per_core] = build_nc(n_per_core)
    return _NC_CACHE[n_per_core]


def _install_ntff_hook():
    """Provide the antenv.axon_hooks shim so trace=True can capture NTFFs."""
    import sys
    if "antenv.axon_hooks" in sys.modules:
        return
    import types
    import ctypes
    import contextlib

    so_path = "/opt/axon/libaxon_pjrt.so"
    lib = ctypes.CDLL(so_path)
    if not hasattr(lib, "axon_start_nrt_profile"):
        return
    lib.axon_start_nrt_profile.argtypes = [ctypes.POINTER(ctypes.c_int64), ctypes.c_size_t]
    lib.axon_start_nrt_profile.restype = ctypes.c_int64
    lib.axon_stop_nrt_profile.argtypes = [ctypes.c_char_p]
    lib.axon_stop_nrt_profile.restype = ctypes.c_int64

    @contextlib.contextmanager
    def _hook(output_dir, device_ids):
        import jax
        jax.devices()
        if device_ids:
            ids = (ctypes.c_int64 * len(device_ids))(*device_ids)
            rc = lib.axon_start_nrt_profile(ids, len(device_ids))
        else:
            rc = lib.axon_start_nrt_profile(None, 0)
        if rc != 0:
            raise RuntimeError(f"axon_start_nrt_profile rc={rc}")
        try:
            yield
        finally:
            n = lib.axon_stop_nrt_profile(str(output_dir).encode())
            print(f"ntff profile: {n} file(s) written to {output_dir}")

    mod = types.ModuleType("antenv.axon_hooks")
    mod.get_axon_ntff_profile_hook = lambda: _hook
    mod.set_axon_ntff_profile_hook = lambda h: None
    sys.modules["antenv.axon_hooks"] = mod


def kernel(x, MU, A, D, PI, trace=False):
    from concourse.bass_utils import run_bass_kernel_spmd
    if trace:
        try:
            _install_ntff_hook()
        except Exception as e:
            print(f"ntff hook install failed: {e}")
            trace = False

    x = np.asarray(x)
    wall, g16, ctile = host_prep(MU, A, D, PI)
    nc = _get_nc()

    in_maps = []
    for c in range(N_CORES):
        packed = pack_core_input(x[c * N_PER_CORE:(c + 1) * N_PER_CORE, :])
        in_maps.append({"xx2": packed, "wall": wall, "g16": g16,
                        "ctile": ctile})

    res = run_bass_kernel_spmd(nc, in_maps, list(range(N_CORES)), trace=trace)
    out = np.concatenate([res.results[c]["out"] for c in range(N_CORES)], axis=0)
    if trace:
        kernel.last_exec_time_ns = res.exec_time_ns
        kernel.last_results = res
    return out
